# revision 1
# baseline (speedup 1.0000x reference)
"""DCMMSR sparse attention TRN2 kernel.

Sharding: 16 (batch, head) pairs -> 8 cores, 2 adjacent heads of one batch
per core (head-parallel). Out-projection is computed per-core as a partial
sum over its 2 heads' feature rows; host sums the 4 partials per batch and
adds bo (the unshard step).

Math notes (B=2, S=512, E=512, H=8, d=64, WSZ=64, TOPK=4, W=8):
 - S % WSZ == 0 so the reference's padding mask is all-true and wts = 1/64.
 - The coarse softmax cancels against the log-bias inside the fine softmax:
     probs = softmax_n( q.k_n * scale + s_{w(n)}/t + (0 if w selected else -inf) )
   where s_w = (1/(64 t?)) ... precisely s_w = sum_{s in w} (q.k_s)^2 /
   (64 * |q|^2 * |k_s|^2), bias uses s_w / t. So no gather and no explicit
   coarse softmax are needed - only the top-4 selection mask.
 - Everything is computed dense over all 512 keys with non-selected windows
   masked to -inf (exp -> exactly 0).
"""

import numpy as np

import concourse.bass as bass
import concourse.mybir as mybir
import concourse.tile as tile
from concourse import bacc
from concourse.bass import ts
from concourse.bass_utils import run_bass_kernel_spmd

F32 = mybir.dt.float32
F32R = mybir.dt.float32r

B, S, E = 2, 512, 512
H, D, WSZ, TOPK = 8, 64, 64, 4
NW = S // WSZ          # 8 windows
NC = 8                 # cores
HPC = 2                # heads per core
D2 = HPC * D           # 128 feature rows per core
P = 128
NCHUNK = S // P        # 4
SCALE = D ** -0.5      # 0.125
NEG = -1.0e9


def r(ap):
    return ap.bitcast(F32R)


def f(ap):
    return ap.bitcast(F32)


def build_kernel(inv64t: float):
    """Build the per-core Tile program. inv64t = 1/(64*t) is baked in."""
    nc = bacc.Bacc(
        "TRN2",
        target_bir_lowering=False,
        debug=False,
        enable_asserts=False,
        num_devices=NC,
    )

    dt_in = {}
    for name, shape in [
        ("xq", [E, S]), ("xk", [E, S]), ("xv", [E, S]),          # x^T per batch
        ("wq", [E, D2]), ("wk", [E, D2]), ("wv", [E, D2]),        # W^T head cols
        ("wo", [D2, E]),                                          # Wo^T head rows
        ("bq", [D2, 1]), ("bk", [D2, 1]), ("bv", [D2, 1]),
        ("eye", [P, P]), ("ones", [P, D]),
        ("sel8", [NW, NCHUNK * P]),                               # [8, 4*128]
        ("bsel", [P, NCHUNK * NW]),                               # [128, 4*8]
    ]:
        dtt = F32 if name in ("bq", "bk", "bv", "xq", "xk", "wq", "wk") else F32R
        dt_in[name] = nc.dram_tensor(name, shape, dtt, kind="ExternalInput").ap()
    out_dram = nc.dram_tensor("out", [S, E], F32, kind="ExternalOutput").ap()

    with tile.TileContext(nc) as tc:
        with (
            tc.tile_pool(name="const", bufs=1) as cpool,
            tc.tile_pool(name="sbig", bufs=4) as sbig,
            tc.tile_pool(name="spt", bufs=6) as spt,
            tc.tile_pool(name="sdsq", bufs=4) as sdsq,
            tc.tile_pool(name="ssm", bufs=4) as ssm,
            tc.tile_pool(name="stiny", bufs=6) as stiny,
            tc.tile_pool(name="pdots", bufs=4, space="PSUM") as pdots,
            tc.tile_pool(name="pbig", bufs=2, space="PSUM") as pbig,
            tc.tile_pool(name="psm", bufs=2, space="PSUM") as psm,
        ):
            # ---- load inputs ----
            def load(name, shape, rearr=None, **kw):
                t = cpool.tile(shape, dt_in[name].dtype, tag=name)
                src = dt_in[name]
                if rearr is not None:
                    src = src.rearrange(rearr, **kw)
                nc.sync.dma_start(out=t[:], in_=src)
                return t

            xq = load("xq", [P, NCHUNK, S], "(c p) t -> p c t", p=P)
            xk = load("xk", [P, NCHUNK, S], "(c p) t -> p c t", p=P)
            xv = load("xv", [P, NCHUNK, S], "(c p) t -> p c t", p=P)
            wq = load("wq", [P, NCHUNK, D2], "(c p) o -> p c o", p=P)
            wk = load("wk", [P, NCHUNK, D2], "(c p) o -> p c o", p=P)
            wv = load("wv", [P, NCHUNK, D2], "(c p) o -> p c o", p=P)
            wo = load("wo", [D, HPC, E], "(h j) o -> j h o", h=HPC)
            bq = load("bq", [D2, 1])
            bk = load("bk", [D2, 1])
            bv = load("bv", [D2, 1])
            eye = load("eye", [P, P])
            ones = load("ones", [P, D])
            sel8 = load("sel8", [NW, NCHUNK * P])
            bsel = load("bsel", [P, NCHUNK * NW])

            # ---- projections (feature-major [o2, token]) ----
            def proj(w_t, x_t, b_t, dt_out, exact=False):
                ps = pbig.tile([P, S], F32, tag="pb")
                for c in range(NCHUNK):
                    nc.tensor.matmul(
                        ps[:], w_t[:, c, :], x_t[:, c, :],
                        start=(c == 0), stop=(c == NCHUNK - 1),
                    )
                sb = cpool.tile([P, S], dt_out, tag=f"f_{w_t.name}")
                if exact:
                    nc.vector.tensor_scalar(
                        sb[:], ps[:], b_t[:], None, op0=mybir.AluOpType.add,
                    )
                else:
                    nc.scalar.activation(
                        sb[:], ps[:], mybir.ActivationFunctionType.Identity,
                        bias=b_t[:], scale=1.0,
                    )
                return sb

            qf = proj(wq, xq, bq, F32, exact=True)     # [128 o2, 512 q]
            kf = proj(wk, xk, bk, F32, exact=True)     # [128 o2, 512 s]
            vf = proj(wv, xv, bv, F32R)     # [128 o2, 512 s]

            # squared features (for norms) on gpsimd (pool engine)
            qsq = cpool.tile([P, S], F32, tag="qsq")
            nc.gpsimd.tensor_mul(qsq[:], qf[:], qf[:])
            ksq = cpool.tile([P, S], F32, tag="ksq")
            nc.gpsimd.tensor_mul(ksq[:], kf[:], kf[:])

            # v token-major with ones column appended per head:
            # v_tok[c] = [128 s, 130]: cols 0:64 h0-v, 64 ones, 65:129 h1-v, 129 ones
            v_tok = []
            for c in range(NCHUNK):
                vt_ps = pbig.tile([P, S], F32R, tag="pb")
                nc.tensor.transpose(r(vt_ps[:, 0:P]), r(vf[:, ts(c, P)]), r(eye[:]))
                vt = cpool.tile([P, 2 * (D + 1)], F32R, tag=f"vtok{c}")
                if c < 2:
                    nc.vector.tensor_copy(vt[:, 0:D], vt_ps[:, 0:D])
                    nc.vector.tensor_copy(vt[:, D + 1:2 * D + 1], vt_ps[:, D:2 * D])
                else:
                    nc.scalar.copy(vt[:, 0:D], vt_ps[:, 0:D])
                    nc.scalar.copy(vt[:, D + 1:2 * D + 1], vt_ps[:, D:2 * D])
                nc.vector.tensor_scalar(
                    vt[:, D:D + 1], vt_ps[:, 0:1], 0.0, 1.0,
                    op0=mybir.AluOpType.mult, op1=mybir.AluOpType.add,
                )
                nc.vector.tensor_scalar(
                    vt[:, 2 * D + 1:2 * D + 2], vt_ps[:, 0:1], 0.0, 1.0,
                    op0=mybir.AluOpType.mult, op1=mybir.AluOpType.add,
                )
                v_tok.append(vt)

            # key norms^-2 per (head, chunk): kn[:, h*4+c] = 1/sum_d k^2
            kn_ps = psm.tile([P, HPC * NCHUNK * NW], F32, tag="ps")
            for h in range(HPC):
                for c in range(NCHUNK):
                    nc.tensor.matmul(
                        kn_ps[:, ts(h * NCHUNK + c, NW)],
                        ksq[h * D:(h + 1) * D, ts(c, P)],
                        f(ones[h * D:(h + 1) * D, 0:NW]),
                    )
            kinv2 = ssm.tile([P, HPC * NCHUNK * NW], F32, tag="kinv2")
            nc.vector.reciprocal(kinv2[:], kn_ps[:])

            attn = []  # per-head normalized attnT [64, 512] tiles
            for h in range(HPC):
                hs = slice(h * D, (h + 1) * D)

                # BselK: window-selector columns scaled by kinv2/(64 t)
                bselk = ssm.tile([P, NCHUNK * NW], F32, tag="bselk")
                for c in range(NCHUNK):
                    nc.vector.tensor_scalar(
                        bselk[:, ts(c, NW)], f(bsel[:, ts(c, NW)]),
                        kinv2[:, (h * NCHUNK + c) * NW:(h * NCHUNK + c) * NW + 1],
                        inv64t,
                        op0=mybir.AluOpType.mult, op1=mybir.AluOpType.mult,
                    )

                # dots^T chunks: [128 s, 512 q] = k_chunk^T q  (all 4 kept live)
                dots = []
                for c in range(NCHUNK):
                    dc = pdots.tile([P, S], F32, tag="dots")
                    nc.tensor.matmul(dc[:], kf[hs, ts(c, P)], qf[hs, :])
                    dots.append(dc)

                # routing scores, q-major: per q-chunk accumulate over s-chunks
                # scores_q[q, w] = sum_s dsq[s, q] * bselk[s, w]
                dsqs = []
                for c in range(NCHUNK):
                    dsq = sdsq.tile([P, S], F32, tag=f"dsq{c}")
                    nc.scalar.activation(
                        dsq[:], dots[c][:], mybir.ActivationFunctionType.Square,
                    )
                    dsqs.append(dsq)
                # query norms^2 [128 q, 4(qc)]
                qn_ps = psm.tile([P, NCHUNK], F32, tag="ps")
                for qc in range(NCHUNK):
                    nc.tensor.matmul(
                        qn_ps[:, qc:qc + 1],
                        qsq[hs, ts(qc, P)],
                        f(ones[hs, 0:1]),
                    )
                qinv2 = stiny.tile([P, NCHUNK], F32, tag="qinv2")
                nc.vector.reciprocal(qinv2[:], qn_ps[:])

                bias_q = ssm.tile([P, NCHUNK * NW], F32, tag="biasq")
                for qc in range(NCHUNK):
                    scq_ps = psm.tile([P, NW], F32, tag="ps")
                    for c in range(NCHUNK):
                        nc.tensor.matmul(
                            scq_ps[:],
                            dsqs[c][:, ts(qc, P)],
                            bselk[:, ts(c, NW)],
                            start=(c == 0), stop=(c == NCHUNK - 1),
                        )
                    scores_q = stiny.tile([P, NW], F32, tag="scq")
                    nc.vector.tensor_scalar(
                        scores_q[:], scq_ps[:], qinv2[:, qc:qc + 1], None,
                        op0=mybir.AluOpType.mult,
                    )
                    srt = stiny.tile([P, 8], F32, tag="srt")
                    nc.vector.max(srt[:], scores_q[:])
                    m = stiny.tile([P, NW], F32, tag="m")
                    nc.gpsimd.tensor_scalar(
                        m[:], scores_q[:],
                        srt[:, TOPK - 1:TOPK], -NEG,
                        op0=mybir.AluOpType.is_ge, op1=mybir.AluOpType.mult,
                    )
                    nc.gpsimd.tensor_add(m[:], m[:], scores_q[:])
                    nc.gpsimd.tensor_scalar(
                        bias_q[:, ts(qc, NW)], m[:], NEG, 1.0 / SCALE,
                        op0=mybir.AluOpType.add, op1=mybir.AluOpType.mult,
                    )

                # transpose bias back to w-major [8, 512]
                bw_ps = psm.tile([NW, S], F32, tag="ps")
                for c in range(NCHUNK):
                    nc.tensor.transpose(
                        bw_ps[:, ts(c, P)], bias_q[:, ts(c, NW)], f(eye[:]),
                    )
                bias_w = ssm.tile([NW, S], F32R, tag="biasw")
                nc.vector.tensor_copy(bias_w[:], bw_ps[:])

                # accumulate expanded bias into dots psum, then exp and PV
                num_ps = pbig.tile([D + 1, S], F32, tag="pb")
                for c in range(NCHUNK):
                    nc.tensor.matmul(
                        dots[c][:], r(sel8[:, ts(c, P)]), r(bias_w[:]),
                        start=False, stop=True, skip_group_check=True,
                    )
                    pt = spt.tile([P, S], F32R, tag="pt")
                    nc.scalar.activation(
                        pt[:], dots[c][:], mybir.ActivationFunctionType.Exp,
                        scale=SCALE,
                    )
                    nc.tensor.matmul(
                        num_ps[:], r(v_tok[c][:, h * (D + 1):(h + 1) * (D + 1)]),
                        r(pt[:]),
                        start=(c == 0), stop=(c == NCHUNK - 1),
                    )

                # normalize: attn_h = numer[0:64] * (1/Z) ; Z = numer row 64
                zinv = stiny.tile([D + 1, S], F32R, tag="zinv")
                with nc.allow_low_precision(reason="fp32r matmul operand"):
                    nc.vector.reciprocal(zinv[D:D + 1, :], num_ps[D:D + 1, :])
                zr_ps = pbig.tile([D, S], F32, tag="pb")
                nc.tensor.matmul(
                    zr_ps[:], r(ones[D:D + 1, 0:D]), r(zinv[D:D + 1, :]),
                )
                num_sb = sbig.tile([D, S], F32, tag="numsb")
                nc.scalar.copy(num_sb[:], num_ps[0:D, :])
                at = sbig.tile([D, S], F32R, tag=f"attn{h}")
                nc.vector.tensor_mul(at[:], num_sb[:], zr_ps[:])
                attn.append(at)

            # ---- partial out-projection [512 t, 512 o] ----
            for c in range(NCHUNK):
                op_ps = pbig.tile([P, E], F32, tag="pb")
                for h in range(HPC):
                    nc.tensor.matmul(
                        op_ps[:], r(attn[h][:, ts(c, P)]), r(wo[:, h, :]),
                        start=(h == 0), stop=(h == HPC - 1),
                    )
                ot = sbig.tile([P, E], F32, tag="osb")
                nc.scalar.copy(ot[:], op_ps[:])
                nc.sync.dma_start(out=out_dram[ts(c, P), :], in_=ot[:])

    nc.compile()
    return nc


_CACHE = {}


def _consts():
    eye = np.eye(P, dtype=np.float32)
    ones = np.ones((P, D), dtype=np.float32)
    # sel8[w, c*128 + s'] = 1 if w == 2c + s'//64
    sel8 = np.zeros((NW, NCHUNK * P), dtype=np.float32)
    for c in range(NCHUNK):
        for sp in range(P):
            sel8[2 * c + sp // WSZ, c * P + sp] = 1.0
    # bsel[s', c*8 + w] = 1 if w == 2c + s'//64
    bsel = np.zeros((P, NCHUNK * NW), dtype=np.float32)
    for c in range(NCHUNK):
        for sp in range(P):
            bsel[sp, c * NW + 2 * c + sp // WSZ] = 1.0
    return eye, ones, sel8, bsel


def kernel(query, key, value, Wq, bq, Wk, bk, Wv, bv, Wo, bo, temp,
           _want_perf=False):
    query = np.asarray(query, dtype=np.float32)
    key = np.asarray(key, dtype=np.float32)
    value = np.asarray(value, dtype=np.float32)
    t = float(np.clip(np.asarray(temp, dtype=np.float32), 0.1, None)[0])
    inv64t = 1.0 / (WSZ * t)

    if inv64t not in _CACHE:
        _CACHE[inv64t] = build_kernel(inv64t)
    nc = _CACHE[inv64t]

    eye, ones, sel8, bsel = _consts()
    WqT = np.ascontiguousarray(np.asarray(Wq, dtype=np.float32).T)
    WkT = np.ascontiguousarray(np.asarray(Wk, dtype=np.float32).T)
    WvT = np.ascontiguousarray(np.asarray(Wv, dtype=np.float32).T)
    WoT = np.ascontiguousarray(np.asarray(Wo, dtype=np.float32).T)
    bq = np.asarray(bq, dtype=np.float32)
    bk = np.asarray(bk, dtype=np.float32)
    bv = np.asarray(bv, dtype=np.float32)
    bo = np.asarray(bo, dtype=np.float32)

    in_maps = []
    for core in range(NC):
        b = core // (NC // B)
        hp = core % (NC // B)
        cols = slice(hp * D2, (hp + 1) * D2)
        in_maps.append({
            "xq": np.ascontiguousarray(query[b].T),
            "xk": np.ascontiguousarray(key[b].T),
            "xv": np.ascontiguousarray(value[b].T),
            "wq": np.ascontiguousarray(WqT[:, cols]),
            "wk": np.ascontiguousarray(WkT[:, cols]),
            "wv": np.ascontiguousarray(WvT[:, cols]),
            "wo": np.ascontiguousarray(WoT[cols, :]),
            "bq": np.ascontiguousarray(bq[cols].reshape(D2, 1)),
            "bk": np.ascontiguousarray(bk[cols].reshape(D2, 1)),
            "bv": np.ascontiguousarray(bv[cols].reshape(D2, 1)),
            "eye": eye, "ones": ones, "sel8": sel8, "bsel": bsel,
        })

    res = run_bass_kernel_spmd(nc, in_maps, core_ids=list(range(NC)),
                               trace=_want_perf)

    out = np.zeros((B, S, E), dtype=np.float32)
    for core in range(NC):
        b = core // (NC // B)
        out[b] += res.results[core]["out"]
    out += bo.reshape(1, 1, E)

    if _want_perf:
        return out, res
    return out



# revision 18
# speedup vs baseline: 1.3060x; 1.3060x over previous
"""DCMMSR sparse attention TRN2 kernel (fp16 fast path).

Sharding: 16 (batch, head) pairs -> 8 cores, 2 adjacent heads of one batch
per core (head-parallel). Out-projection is computed per-core as a partial
sum over its 2 heads' feature rows; host sums the 4 partials per batch and
adds bo (the unshard step).

Math notes (B=2, S=512, E=512, H=8, d=64, WSZ=64, TOPK=4, W=8):
 - S % WSZ == 0 so the reference's padding mask is all-true and wts = 1/64.
 - The coarse softmax cancels against the log-bias inside the fine softmax
   up to a per-query constant:
     probs = softmax_n( q.k_n * scale + s_{w(n)}/t + (0 if w sel else -inf) )
   where s_w = sum_{s in w} (q.k_s)^2 / (64 t |q|^2 |k_s|^2). So no gather
   and no explicit coarse softmax - only the top-4 selection mask, applied
   densely over all 512 keys with non-selected windows biased to -200
   (exp -> exactly 0, while keeping the selected windows' s_w bias exact
   to ~3e-5: (200 + s) - 200 preserves s at f32 ulp(256)).
 - q is pre-scaled by d^-0.5 at projection time; scores are invariant
   because qinv2 is computed from the scaled q (exact cancellation), and
   the fine logits then need no further scaling.
"""

import numpy as np

import concourse.bass as bass
import concourse.mybir as mybir
import concourse.tile as tile
from concourse import bacc
from concourse.bass import ts
from concourse.bass_utils import run_bass_kernel_spmd

F32 = mybir.dt.float32
F16 = mybir.dt.float16

B, S, E = 2, 512, 512
H, D, WSZ, TOPK = 8, 64, 64, 4
NW = S // WSZ          # 8 windows
NC = 8                 # cores
HPC = 2                # heads per core
D2 = HPC * D           # 128 feature rows per core
P = 128
NCHUNK = S // P        # 4
SCALE = D ** -0.5      # 0.125
SENT = 200.0           # selection sentinel; exp(x-200) flushes to 0


def build_kernel(inv64t: float, fast: bool):
    """Per-core Tile program. inv64t = 1/(64*t) is baked in.

    fast=True : all-fp16 matmul path (score path sees fp16-rounded inputs).
    fast=False: q/k/dots/score path kept in fp32; v/attn path fp16.
    """
    nc = bacc.Bacc(
        "TRN2",
        target_bir_lowering=False,
        debug=False,
        enable_asserts=False,
        num_devices=NC,
    )

    DT_QK = F16 if fast else F32   # qf/kf storage -> dots operand dtype
    DT_SC = F16 if fast else F32   # dsq / bselk -> score matmul dtype

    dt_in = {}
    for name, shape, dtt in [
        ("xq", [E, S], F16), ("xk", [E, S], F16), ("xv", [E, S], F16),
        ("wq", [E, D2], F16), ("wk", [E, D2], F16), ("wv", [E, D2], F16),
        ("wo", [D2, E], F16),
        ("bias3", [D2, 4], F32),                 # bq*SCALE, bk, bv, ones cols
        ("blob", [P, P + D + NCHUNK * NW], F16),  # eye | ones | bsel
        ("sel8", [NW, NCHUNK * P], F16),
    ]:
        dt_in[name] = nc.dram_tensor(name, shape, dtt, kind="ExternalInput").ap()
    out_dram = nc.dram_tensor("out", [S, E], F16, kind="ExternalOutput").ap()

    with tile.TileContext(nc) as tc, nc.allow_low_precision(reason="fp16 path"):
        with (
            tc.tile_pool(name="const", bufs=1) as cpool,
            tc.tile_pool(name="sbig", bufs=4) as sbig,
            tc.tile_pool(name="spt", bufs=6) as spt,
            tc.tile_pool(name="sdsq", bufs=8) as sdsq,
            tc.tile_pool(name="ssm", bufs=4) as ssm,
            tc.tile_pool(name="stiny", bufs=6) as stiny,
            tc.tile_pool(name="pdots", bufs=4, space="PSUM") as pdots,
            tc.tile_pool(name="pbig", bufs=2, space="PSUM") as pbig,
            tc.tile_pool(name="psm", bufs=2, space="PSUM") as psm,
        ):
            # ---- load inputs (priority order; consts off the sync queue) ----
            def load(name, shape, rearr=None, eng=None, **kw):
                t = cpool.tile(shape, dt_in[name].dtype, tag=name)
                src = dt_in[name]
                if rearr is not None:
                    src = src.rearrange(rearr, **kw)
                (eng or nc.sync).dma_start(out=t[:], in_=src)
                return t

            bias3 = load("bias3", [D2, 4], eng=nc.gpsimd)
            blob = load("blob", [P, P + D + NCHUNK * NW], eng=nc.gpsimd)
            sel8 = load("sel8", [NW, NCHUNK * P], eng=nc.gpsimd)
            eye = blob[:, 0:P]
            ones = blob[:, P:P + D]
            bsel = blob[:, P + D:]

            wq = load("wq", [P, NCHUNK, D2], "(c p) o -> p c o", p=P)
            xq = load("xq", [P, NCHUNK, S], "(c p) t -> p c t", p=P)
            wk = load("wk", [P, NCHUNK, D2], "(c p) o -> p c o", p=P)
            xk = load("xk", [P, NCHUNK, S], "(c p) t -> p c t", p=P)
            wv = load("wv", [P, NCHUNK, D2], "(c p) o -> p c o", p=P)
            xv = load("xv", [P, NCHUNK, S], "(c p) t -> p c t", p=P)
            wo = load("wo", [P, E], eng=nc.scalar)

            # ---- projections (feature-major [o2, token]) ----
            def proj(w_t, x_t, bcol, dt_out, scale):
                ps = pbig.tile([P, S], F32, tag="pb")
                for c in range(NCHUNK):
                    nc.tensor.matmul(
                        ps[:], w_t[:, c, :], x_t[:, c, :],
                        start=(c == 0), stop=(c == NCHUNK - 1),
                    )
                sb = cpool.tile([P, S], dt_out, tag=f"f_{w_t.name}")
                nc.vector.tensor_scalar(
                    sb[:], ps[:], scale, bias3[:, bcol:bcol + 1],
                    op0=mybir.AluOpType.mult, op1=mybir.AluOpType.add,
                )
                return sb

            qf = proj(wq, xq, 0, DT_QK, SCALE)   # [128 o2, 512 q], pre-scaled
            kf = proj(wk, xk, 1, DT_QK, 1.0)     # [128 o2, 512 s]
            vf = proj(wv, xv, 2, F16, 1.0)

            # squared features (for norms) on gpsimd
            qsq = cpool.tile([P, S], DT_SC, tag="qsq")
            nc.gpsimd.tensor_mul(qsq[:], qf[:], qf[:])
            ksq = cpool.tile([P, S], DT_SC, tag="ksq")
            nc.gpsimd.tensor_mul(ksq[:], kf[:], kf[:])

            # v token-major with ones column appended per head:
            # v_tok[c] = [128 s, 130]: cols 0:64 h0-v, 64 ones, 65:129 h1-v,
            # 129 ones
            v_tok = []
            for c in range(NCHUNK):
                vt_ps = pbig.tile([P, 2 * S], F16, tag="pb")
                nc.tensor.transpose(vt_ps[:, 0:P], vf[:, ts(c, P)], eye)
                vt = cpool.tile([P, 2 * (D + 1)], F16, tag=f"vtok{c}")
                if c < 2:
                    nc.vector.tensor_copy(vt[:, 0:D], vt_ps[:, 0:D])
                    nc.vector.tensor_copy(vt[:, D + 1:2 * D + 1], vt_ps[:, D:2 * D])
                else:
                    nc.scalar.copy(vt[:, 0:D], vt_ps[:, 0:D])
                    nc.scalar.copy(vt[:, D + 1:2 * D + 1], vt_ps[:, D:2 * D])
                nc.vector.tensor_scalar(
                    vt[:, D:D + 1], vt_ps[:, 0:1], 0.0, 1.0,
                    op0=mybir.AluOpType.mult, op1=mybir.AluOpType.add,
                )
                nc.vector.tensor_scalar(
                    vt[:, 2 * D + 1:2 * D + 2], vt_ps[:, 0:1], 0.0, 1.0,
                    op0=mybir.AluOpType.mult, op1=mybir.AluOpType.add,
                )
                v_tok.append(vt)

            # key/query norms^-2 per (head, chunk): col h*4+c
            kn_ps = psm.tile([P, HPC * NCHUNK], F32, tag="ps")
            qn_ps = psm.tile([P, HPC * NCHUNK], F32, tag="ps")
            for h in range(HPC):
                hs = slice(h * D, (h + 1) * D)
                onecol = ones[hs, 0:1] if fast else bias3[hs, 3:4]
                for c in range(NCHUNK):
                    col = h * NCHUNK + c
                    nc.tensor.matmul(
                        kn_ps[:, col:col + 1], ksq[hs, ts(c, P)], onecol,
                        skip_group_check=True,
                    )
                    nc.tensor.matmul(
                        qn_ps[:, col:col + 1], qsq[hs, ts(c, P)], onecol,
                        skip_group_check=True,
                    )
            kinv2 = stiny.tile([P, HPC * NCHUNK], F32, tag="kinv2")
            nc.vector.reciprocal(kinv2[:], kn_ps[:])
            qinv2 = stiny.tile([P, HPC * NCHUNK], F32, tag="qinv2")
            nc.vector.reciprocal(qinv2[:], qn_ps[:])

            at = cpool.tile([P, S], F16, tag="attn")  # both heads stacked
            for h in range(HPC):
                hs = slice(h * D, (h + 1) * D)

                # window-selector columns scaled by kinv2/(64 t)
                bselk = ssm.tile([P, NCHUNK * NW], DT_SC, tag="bselk")
                for c in range(NCHUNK):
                    nc.vector.tensor_scalar(
                        bselk[:, ts(c, NW)], bsel[:, ts(c, NW)],
                        kinv2[:, h * NCHUNK + c:h * NCHUNK + c + 1],
                        inv64t,
                        op0=mybir.AluOpType.mult, op1=mybir.AluOpType.mult,
                    )

                # dots^T chunks: [128 s, 512 q] = k_chunk^T q (4 kept live)
                dots = []
                for c in range(NCHUNK):
                    dc = pdots.tile([P, S], F32, tag="dots")
                    nc.tensor.matmul(dc[:], kf[hs, ts(c, P)], qf[hs, :])
                    dots.append(dc)

                dsqs = []
                for c in range(NCHUNK):
                    dsq = sdsq.tile([P, S], DT_SC, tag=f"dsq{c}")
                    nc.scalar.activation(
                        dsq[:], dots[c][:],
                        mybir.ActivationFunctionType.Square,
                    )
                    dsqs.append(dsq)

                # routing scores, q-major, then top-4 mask and bias
                bias_q = ssm.tile([P, NCHUNK * NW], F16, tag="biasq")
                for qc in range(NCHUNK):
                    scq_ps = psm.tile([P, NW], F32, tag="ps")
                    for c in range(NCHUNK):
                        nc.tensor.matmul(
                            scq_ps[:],
                            dsqs[c][:, ts(qc, P)],
                            bselk[:, ts(c, NW)],
                            start=(c == 0), stop=(c == NCHUNK - 1),
                        )
                    scores_q = stiny.tile([P, NW], F32, tag="scq")
                    nc.vector.tensor_scalar(
                        scores_q[:], scq_ps[:],
                        qinv2[:, h * NCHUNK + qc:h * NCHUNK + qc + 1], None,
                        op0=mybir.AluOpType.mult,
                    )
                    srt = stiny.tile([P, 8], F32, tag="srt")
                    nc.vector.max(srt[:], scores_q[:])
                    m = stiny.tile([P, NW], F32, tag="m")
                    nc.gpsimd.tensor_scalar(
                        m[:], scores_q[:],
                        srt[:, TOPK - 1:TOPK], SENT,
                        op0=mybir.AluOpType.is_ge, op1=mybir.AluOpType.mult,
                    )
                    nc.gpsimd.tensor_add(m[:], m[:], scores_q[:])
                    nc.gpsimd.tensor_scalar(
                        bias_q[:, ts(qc, NW)], m[:], -SENT, None,
                        op0=mybir.AluOpType.add,
                    )

                # transpose bias back to w-major [8, 512]
                bw_ps = psm.tile([NW, S], F16, tag="ps")
                for c in range(NCHUNK):
                    nc.tensor.transpose(
                        bw_ps[:, ts(c, P)], bias_q[:, ts(c, NW)], eye,
                    )
                bias_w = ssm.tile([NW, S], F16, tag="biasw")
                nc.vector.tensor_copy(bias_w[:], bw_ps[:])

                # accumulate expanded bias into dots psum, then exp and PV
                num_ps = pbig.tile([D + 1, S], F32, tag="pb")
                for c in range(NCHUNK):
                    nc.tensor.matmul(
                        dots[c][:], sel8[:, ts(c, P)], bias_w[:],
                        start=False, stop=True, skip_group_check=True,
                    )
                    pt = spt.tile([P, S], F16, tag="pt")
                    nc.scalar.activation(
                        pt[:], dots[c][:], mybir.ActivationFunctionType.Exp,
                    )
                    nc.tensor.matmul(
                        num_ps[:], v_tok[c][:, h * (D + 1):(h + 1) * (D + 1)],
                        pt[:],
                        start=(c == 0), stop=(c == NCHUNK - 1),
                    )

                # normalize: at[hs] = numer[0:64] * (1/Z); Z = numer row 64
                zinv = stiny.tile([D + 1, S], F16, tag="zinv")
                nc.vector.reciprocal(zinv[D:D + 1, :], num_ps[D:D + 1, :])
                zr_ps = pbig.tile([D, S], F32, tag="pb")
                nc.tensor.matmul(zr_ps[:], ones[D:D + 1, 0:D], zinv[D:D + 1, :])
                num_sb = sbig.tile([D, S], F32, tag="numsb")
                nc.vector.tensor_copy(num_sb[:], num_ps[0:D, :])
                nc.vector.tensor_mul(at[hs, :], num_sb[:], zr_ps[:])

            # ---- partial out-projection [512 t, 512 o], heads fused ----
            for c in range(NCHUNK):
                op_ps = pbig.tile([P, E], F32, tag="pb")
                nc.tensor.matmul(op_ps[:], at[:, ts(c, P)], wo[:])
                ot = sbig.tile([P, E], F16, tag="osb")
                if c % 2 == 0:
                    nc.scalar.copy(ot[:], op_ps[:])
                else:
                    nc.vector.tensor_copy(ot[:], op_ps[:])
                nc.sync.dma_start(out=out_dram[ts(c, P), :], in_=ot[:])

    nc.compile()
    return nc


_CACHE = {}


def _consts():
    eye = np.eye(P, dtype=np.float16)
    ones = np.ones((P, D), dtype=np.float16)
    # sel8[w, c*128 + s'] = 1 if w == 2c + s'//64
    sel8 = np.zeros((NW, NCHUNK * P), dtype=np.float16)
    for c in range(NCHUNK):
        for sp in range(P):
            sel8[2 * c + sp // WSZ, c * P + sp] = 1.0
    # bsel[s', c*8 + w] = 1 if w == 2c + s'//64
    bsel = np.zeros((P, NCHUNK * NW), dtype=np.float16)
    for c in range(NCHUNK):
        for sp in range(P):
            bsel[sp, c * NW + 2 * c + sp // WSZ] = 1.0
    blob = np.concatenate([eye, ones, bsel], axis=1)
    return blob, sel8


def kernel(query, key, value, Wq, bq, Wk, bk, Wv, bv, Wo, bo, temp,
           _want_perf=False, _fast=True):
    query = np.asarray(query, dtype=np.float32)
    key = np.asarray(key, dtype=np.float32)
    value = np.asarray(value, dtype=np.float32)
    t = float(np.clip(np.asarray(temp, dtype=np.float32), 0.1, None)[0])
    inv64t = 1.0 / (WSZ * t)

    ck = (inv64t, _fast)
    if ck not in _CACHE:
        _CACHE[ck] = build_kernel(inv64t, _fast)
    nc = _CACHE[ck]

    blob, sel8 = _consts()
    WqT = np.asarray(Wq, dtype=np.float32).T.astype(np.float16)
    WkT = np.asarray(Wk, dtype=np.float32).T.astype(np.float16)
    WvT = np.asarray(Wv, dtype=np.float32).T.astype(np.float16)
    WoT = np.asarray(Wo, dtype=np.float32).T.astype(np.float16)
    bqs = np.asarray(bq, dtype=np.float32) * SCALE
    bk = np.asarray(bk, dtype=np.float32)
    bv = np.asarray(bv, dtype=np.float32)
    bo = np.asarray(bo, dtype=np.float32)
    x16 = {}
    for nm, arr in (("q", query), ("k", key), ("v", value)):
        for b in range(B):
            x16[nm, b] = np.ascontiguousarray(arr[b].T.astype(np.float16))

    in_maps = []
    for core in range(NC):
        b = core // (NC // B)
        hp = core % (NC // B)
        cols = slice(hp * D2, (hp + 1) * D2)
        bias3 = np.stack([bqs[cols], bk[cols], bv[cols],
                          np.ones(D2, dtype=np.float32)], axis=1)
        in_maps.append({
            "xq": x16["q", b],
            "xk": x16["k", b],
            "xv": x16["v", b],
            "wq": np.ascontiguousarray(WqT[:, cols]),
            "wk": np.ascontiguousarray(WkT[:, cols]),
            "wv": np.ascontiguousarray(WvT[:, cols]),
            "wo": np.ascontiguousarray(WoT[cols, :]),
            "bias3": np.ascontiguousarray(bias3),
            "blob": blob, "sel8": sel8,
        })

    res = run_bass_kernel_spmd(nc, in_maps, core_ids=list(range(NC)),
                               trace=_want_perf)

    out = np.zeros((B, S, E), dtype=np.float32)
    for core in range(NC):
        b = core // (NC // B)
        out[b] += res.results[core]["out"].astype(np.float32)
    out += bo.reshape(1, 1, E)

    if _want_perf:
        return out, res
    return out
